# revision 3
# baseline (speedup 1.0000x reference)
"""Trainium2 Bass kernel for nn_Attention_63934883168998.

Math (per token t): q,k,v = x W{q,k,v}^T reshaped (16 heads, 64); scores over
HEADS: S = q k^T / 8 (16x16), A = softmax(S), out = A v -> (1024); y = out Wo^T.

Sharding: pure data parallel over the 16384 tokens -> 2048 tokens/core.
On-chip in fp16 (PE fp16 matmul = full rate; fp16 keeps GEMM rel-err ~5e-4).
Projections on PE: stationary = x^T chunk (via DMA transpose), moving = host-
pretransposed W^T (h,o).  The per-token 16x16 head attention runs on the
Vector engine via broadcast-AP multiplies + segmented reduces; softmax exp on
ScalarE.
"""

import numpy as np

N_CORES = 8
HID = 1024
NH, HD = 16, 64
TILE = 128
TPC = 16384 // N_CORES      # tokens per core
NT = TPC // TILE            # token tiles per core
NC_CHUNK = HID // 128       # 8 hidden chunks

_cache = {}


def _build():
    if "nc" in _cache:
        return
    import concourse.bacc as bacc
    import concourse.mybir as mybir
    from concourse import tile

    f16 = mybir.dt.float16
    f32 = mybir.dt.float32
    AX = mybir.AxisListType
    OP = mybir.AluOpType
    AF = mybir.ActivationFunctionType

    nc = bacc.Bacc("TRN2", target_bir_lowering=False, debug=False)
    xs = nc.dram_tensor("xs", (TPC, HID), f16, kind="ExternalInput").ap()
    wts = {
        n: nc.dram_tensor(n, (HID, HID), f16, kind="ExternalInput").ap()
        for n in ("wqt", "wkt", "wvt", "wot")
    }
    y = nc.dram_tensor("y", (TPC, HID), f32, kind="ExternalOutput").ap()

    with tile.TileContext(nc) as tc:
        with (
            tc.tile_pool(name="wpool", bufs=1) as wpool,
            tc.tile_pool(name="work", bufs=2) as work,
            tc.tile_pool(name="prod", bufs=1) as prodp,
            tc.tile_pool(name="psum", bufs=1, space="PSUM") as pp,
        ):
            # Resident weights, laid out (128, chunk, out) so chunk c is
            # W^T[c*128:(c+1)*128, :] with hidden-in on partitions.
            w_sb = {}
            for n in ("wqt", "wkt", "wvt", "wot"):
                wt = wpool.tile([128, NC_CHUNK, HID], f16, tag=n)
                nc.sync.dma_start(wt[:], wts[n].rearrange("(c p) o -> p c o", p=128))
                w_sb[n] = wt

            for it in range(NT):
                t0 = it * TILE
                # x^T chunks: (h_chunk 128, tokens 128) each, via DMA transpose
                xT = work.tile([128, NC_CHUNK, TILE], f16, tag="xT")
                for c in range(NC_CHUNK):
                    nc.sync.dma_start(
                        xT[:, c, :],
                        xs[t0 : t0 + TILE, c * 128 : (c + 1) * 128],
                        transpose=True,
                    )

                # q,k,v projections: psum[t, o_half] += xT_c^T @ W^T[c, half]
                ps = {
                    n: [pp.tile([128, 512], f32, name=f"ps{n}{h}", tag=f"ps{n}{h}") for h in range(2)]
                    for n in ("q", "k", "v")
                }
                for c in range(NC_CHUNK):
                    for n, wn in (("q", "wqt"), ("k", "wkt"), ("v", "wvt")):
                        for h in range(2):
                            nc.tensor.matmul(
                                ps[n][h][:],
                                xT[:, c, :],
                                w_sb[wn][:, c, h * 512 : (h + 1) * 512],
                                start=(c == 0),
                                stop=(c == NC_CHUNK - 1),
                            )

                q_sb = work.tile([128, HID], f16, tag="q")
                k_sb = work.tile([128, HID], f16, tag="k")
                # v stored d-major: (128, d 64, g 16) for the AV stage
                v_pm = work.tile([128, HD, NH], f16, tag="v")
                for h in range(2):
                    nc.scalar.copy(q_sb[:, h * 512 : (h + 1) * 512], ps["q"][h][:])
                    nc.scalar.copy(k_sb[:, h * 512 : (h + 1) * 512], ps["k"][h][:])
                    # psum v half h holds heads g=8h..8h+8 (g-major (g,d));
                    # write transposed into (d, g) layout
                    dst = v_pm[:, :, h * 8 : (h + 1) * 8]  # (128, 64, 8) strides (16,1)
                    src = ps["v"][h][:].rearrange("p (g d) -> p g d", g=8)
                    nc.scalar.copy(dst.rearrange("p d g -> p g d"), src)

                # scores: prod[t,(h,g,d)] = q[t,(h,d)] * k[t,(g,d)]; reduce d
                prod = prodp.tile([128, NH, NH, HD], f16, tag="prod")
                q_ap = (
                    q_sb[:]
                    .rearrange("p (h d) -> p h d", h=NH)
                    .unsqueeze(2)
                    .broadcast_to((128, NH, NH, HD))
                )
                k_ap = (
                    k_sb[:]
                    .rearrange("p (g d) -> p g d", g=NH)
                    .unsqueeze(1)
                    .broadcast_to((128, NH, NH, HD))
                )
                nc.vector.tensor_tensor(prod[:], q_ap, k_ap, op=OP.mult)
                scores = work.tile([128, NH, NH], f32, tag="scores")
                nc.vector.tensor_reduce(scores[:], prod[:], axis=AX.X, op=OP.add)

                # softmax over g (no max-subtract: logits ~N(0,1), exp safe)
                ex = work.tile([128, NH, NH], f16, tag="ex")
                nc.scalar.activation(ex[:], scores[:], AF.Exp, scale=0.125)
                ssum = work.tile([128, NH], f32, tag="ssum")
                nc.vector.tensor_reduce(ssum[:], ex[:], axis=AX.X, op=OP.add)
                rs = work.tile([128, NH], f32, tag="rs")
                nc.vector.reciprocal(rs[:], ssum[:])
                attw = work.tile([128, NH, NH], f16, tag="attw")
                nc.vector.tensor_tensor(
                    attw[:], ex[:], rs[:].unsqueeze(2).broadcast_to((128, NH, NH)),
                    op=OP.mult,
                )

                # AV: prod2[t,(h,d,g)] = A[t,(h,g)] * v[t,(d,g)]; reduce g
                prod2 = prodp.tile([128, NH, HD, NH], f16, tag="prod")
                a_ap = attw[:].unsqueeze(2).broadcast_to((128, NH, HD, NH))
                v_ap = v_pm[:].unsqueeze(1).broadcast_to((128, NH, HD, NH))
                nc.vector.tensor_tensor(prod2[:], a_ap, v_ap, op=OP.mult)
                attn = work.tile([128, NH, HD], f32, tag="attn")
                nc.vector.tensor_reduce(attn[:], prod2[:], axis=AX.X, op=OP.add)
                attn16 = work.tile([128, HID], f16, tag="attn16")
                nc.scalar.copy(
                    attn16[:].rearrange("p (h d) -> p h d", h=NH), attn[:]
                )

                # output projection: oT chunks via SBUF->SBUF DMA transpose
                oT = work.tile([128, NC_CHUNK, TILE], f16, tag="oT")
                for c in range(NC_CHUNK):
                    nc.sync.dma_start(
                        oT[:, c, :], attn16[:, c * 128 : (c + 1) * 128], transpose=True
                    )
                py = [pp.tile([128, 512], f32, name=f"py{h}", tag=f"py{h}") for h in range(2)]
                for c in range(NC_CHUNK):
                    for h in range(2):
                        nc.tensor.matmul(
                            py[h][:],
                            oT[:, c, :],
                            w_sb["wot"][:, c, h * 512 : (h + 1) * 512],
                            start=(c == 0),
                            stop=(c == NC_CHUNK - 1),
                        )
                y_sb = work.tile([128, HID], f32, tag="ysb")
                for h in range(2):
                    nc.scalar.copy(y_sb[:, h * 512 : (h + 1) * 512], py[h][:])
                nc.sync.dma_start(y[t0 : t0 + TILE, :], y_sb[:])

    nc.compile()
    _cache["nc"] = nc


def _prep_inputs(x, wq, wk, wv, wo):
    x2 = np.asarray(x, dtype=np.float32).reshape(-1, HID)
    w16 = {
        n: np.ascontiguousarray(np.asarray(w, dtype=np.float32).T).astype(np.float16)
        for n, w in (("wqt", wq), ("wkt", wk), ("wvt", wv), ("wot", wo))
    }
    in_maps = []
    for i in range(N_CORES):
        sh = x2[i * TPC : (i + 1) * TPC].astype(np.float16)
        m = {"xs": np.ascontiguousarray(sh)}
        m.update(w16)
        in_maps.append(m)
    return in_maps


def kernel(x, wq, wk, wv, wo, _trace=False):
    from concourse import bass_utils

    _build()
    in_maps = _prep_inputs(x, wq, wk, wv, wo)
    res = bass_utils.run_bass_kernel_spmd(
        _cache["nc"], in_maps, core_ids=list(range(N_CORES)), trace=_trace
    )
    kernel.last_result = res
    B, S = 4, 4096
    out = np.concatenate([r["y"] for r in res.results], axis=0)
    return out.reshape(B, S, HID).astype(np.float32)
